# revision 4
# baseline (speedup 1.0000x reference)
"""GCNContext GNN kernel for 8 TRN2 NeuronCores (Bass/Tile, SPMD).

Reference computation (see harness):
    x1 = relu(SAGE(emb; Wl1,bl1,Wr1));  x2 = SAGE(x1; Wl2,bl2,Wr2)
    x  = x2 + emb
    emd = [sum_l x[sentence], sum_l x[context]]  -> BatchNorm -> MLP -> [B,2]

Distribution: nodes+edges sharded by dst core (6250/core), MLP head
replicated, batch rows data-parallel (512/core).

v2 design (segment-matmul aggregation — no DMA scatter):
  * segment-sum of x[src] over dst: GPSIMD dma_gather pulls edge src rows
    (bf16, 256B packets) into SBUF in dst-chunk-grouped order; per
    128-token block a one-hot matrix (DVE iota+is_equal, bf16) selects
    each token's dst row, and PE matmuls accumulate agg[dst(128), feat]
    in PSUM across the chunk's blocks. Replaces the v1 dma_scatter_add
    chains (1.5ms serial GPSIMD desc-gen + 200k RMW descriptors) and the
    bf16->f32 CAST of the gathered stream.
  * conv2 pre-multiply: z = x1 @ Wl2 is computed in the conv1 dense loop
    (reusing the x1^T transposes), AllGathered bf16 [50008,128], and
    aggregated instead of x1 — mean2 @ Wl2 == (Adj z)/cnt. Halves conv2
    gather bytes and the inter-conv collective vs gathering x1 (256-wide).
  * edges are grouped into pieces of 8 dst chunks; each piece issues two
    gathers (lo/hi int16 table halves) on rotating SWDGE queues (4) with
    pool bufs=2 double buffering, so desc-gen/DMA of piece p+1 overlap
    PE/DVE of piece p. Chunk boundaries inside a piece are not
    128-aligned; boundary blocks are matmul'd into both chunks (out-of-
    window dst ids fail the base-shifted is_equal, adding zero).
  * in-degree counts from host metadata; readout via pair-packed bf16
    x_pad view + parity copy_predicated + strided L-reduction (as v1);
    BatchNorm stats AllReduced; MLP replicated per 512-row batch shard.

Perf history (HW exec, NTFF): 7.74ms f32 scatter baseline -> ... ->
5.33ms v1 best (bf16 tables, 2 SWDGE queues, pair-packed readout).
"""
import sys

sys.path.insert(0, "/opt/trn_rl_repo")

import numpy as np

import concourse.bacc as bacc
import concourse.bass as bass
import concourse.mybir as mybir
import concourse.tile as tile
from concourse.bass_utils import run_bass_kernel_spmd
from concourse.masks import make_identity

NCORES = 8
N, D, H, B, L = 50000, 128, 256, 4096, 50
SH = N // NCORES          # 6250 nodes per shard
BSH = B // NCORES         # 512 batch rows per core
LOSPLIT = 25000           # node-id split for int16 gather tables (conv1)
SHP = SH + 1              # padded shard rows (zero row at 6250)
NP_ = NCORES * SHP        # 50008 padded table rows
PADLO = (NCORES // 2) * SHP   # 25004: row split of the padded tables
NM = (SH + 127) // 128    # 49 dst chunks per core (last has 106 rows)
PCH = 8                   # dst chunks per gather piece
NPC = (NM + PCH - 1) // PCH
EPS = 1e-5
F32 = mybir.dt.float32
BF16 = mybir.dt.bfloat16
I16 = mybir.dt.int16

_cache = {}


def _wrap_idx(a):
    """1-D int array (len % 16 == 0) -> [128, n/16] int16 wrapped layout."""
    a16 = np.asarray(a, np.int64).reshape(-1, 16).T.astype(np.int16)
    return np.tile(a16, (8, 1))


def _padmap(n):
    """node id -> row in the padded (zero-row-per-shard) tables."""
    return (n // SH) * SHP + (n % SH)


def _ceil128(x):
    return (int(x) + 127) // 128 * 128


def _plan_edges(src, dst):
    """Group edges per core into (piece, lo/hi-half) streams.

    Streams are ordered by (dst chunk, src) inside each half; budgets
    (max over cores, ceil128) and per-chunk block ranges are shared by
    the SPMD program; per-core shortfall is padded with dst=-1 tokens.

    Returns (budgets, blkrng, percore):
      budgets[p] = (lo_b, hi_b) tokens, each a multiple of 128
      blkrng[m] = (ls, le, hs, he) block ranges in piece m//PCH's tile
      percore[c][p][h] = (src_ids, dst_local) sorted by (chunk, src)
    """
    core = dst // SH
    percore = []
    # cum[c][p][h]: per-chunk token start offsets within the stream
    starts = np.zeros((NCORES, NPC, 2, PCH + 1), np.int64)
    for c in range(NCORES):
        msk = core == c
        s_c, ld = src[msk], dst[msk] - c * SH
        ch = ld // 128
        pc = ch // PCH
        lo = s_c < LOSPLIT
        plist = []
        for p in range(NPC):
            halves = []
            for h, hm in enumerate((lo, ~lo)):
                sel = (pc == p) & hm
                ss, dd, cc = s_c[sel], ld[sel], ch[sel]
                o = np.lexsort((ss, cc))
                ss, dd, cc = ss[o], dd[o], cc[o]
                halves.append((ss, dd))
                for j, m in enumerate(range(p * PCH,
                                            min((p + 1) * PCH, NM))):
                    starts[c, p, h, j + 1] = starts[c, p, h, j] + int(
                        (cc == m).sum())
                starts[c, p, h, PCH] = len(ss)
            plist.append(halves)
        percore.append(plist)

    budgets = []
    for p in range(NPC):
        lo_b = _ceil128(max(len(percore[c][p][0][0]) for c in range(NCORES)))
        hi_b = _ceil128(max(len(percore[c][p][1][0]) for c in range(NCORES)))
        budgets.append((lo_b, hi_b))

    blkrng = []
    for m in range(NM):
        p, j = m // PCH, m % PCH
        lo_b = budgets[p][0]
        ls = int(starts[:, p, 0, j].min()) // 128
        le = -(-int(starts[:, p, 0, j + 1].max()) // 128)
        hs = lo_b // 128 + int(starts[:, p, 1, j].min()) // 128
        he = lo_b // 128 - (-int(starts[:, p, 1, j + 1].max()) // 128)
        # clip to the piece (ceil128 budget may exceed last chunk's end)
        le = min(le, lo_b // 128)
        he = min(he, (lo_b + budgets[p][1]) // 128)
        assert ls < le or hs < he, f"empty chunk {m}"
        blkrng.append((ls, le, hs, he))
    return budgets, blkrng, percore


def _readout_idx(tok):
    """[BSH, L] padded-table row ids -> pair-packed idx + parity mask."""
    nblk = BSH // 128
    m = tok.reshape(nblk, 128, L).transpose(0, 2, 1)       # [blk, l, p]
    m = m.reshape(nblk, 2, L // 2, 128)                    # [blk, h, lp, p]
    idx = (m // 2).reshape(-1)
    par = (m % 2).astype(np.int8)
    par_t = np.ascontiguousarray(
        par.transpose(3, 0, 1, 2).reshape(128, nblk * L))  # [p, blk*50+h*25+lp]
    return _wrap_idx(idx), par_t


def _prepare(inputs):
    src = np.asarray(inputs["edge_index"][0], np.int64)
    dst = np.asarray(inputs["edge_index"][1], np.int64)
    emb = np.asarray(inputs["emb"], np.float32)

    budgets, blkrng, percore = _plan_edges(src, dst)
    ttot = sum(lo + hi for lo, hi in budgets)

    import ml_dtypes
    gab = emb.astype(ml_dtypes.bfloat16)

    sent = np.asarray(inputs["sentence"], np.int64)
    cont = np.asarray(inputs["context"], np.int64)
    core_arr = dst // SH

    in_maps = []
    for c in range(NCORES):
        g1 = np.zeros(ttot, np.int64)
        g2 = np.zeros(ttot, np.int64)
        dv = np.full(ttot, -1.0, np.float32)
        pos = 0
        for p in range(NPC):
            for h in range(2):
                ss, dd = percore[c][p][h]
                n = len(ss)
                if h == 0:
                    g1[pos:pos + n] = ss
                    g2[pos:pos + n] = _padmap(ss)
                else:
                    g1[pos:pos + n] = ss - LOSPLIT
                    g2[pos:pos + n] = _padmap(ss) - PADLO
                dv[pos:pos + n] = dd
                pos += budgets[p][h]
        assert pos == ttot

        rs, rs_par = _readout_idx(_padmap(sent[c * BSH:(c + 1) * BSH]))
        rc, rc_par = _readout_idx(_padmap(cont[c * BSH:(c + 1) * BSH]))

        deg = np.bincount(dst[core_arr == c] - c * SH,
                          minlength=SH).astype(np.float32)
        sl = slice(c * SH, (c + 1) * SH)
        in_maps.append({
            "cnt_in": deg.reshape(SH, 1),
            "gab": gab,
            "eloc": emb[sl].copy(),
            "elocT": np.ascontiguousarray(emb[sl].T),
            "g1": _wrap_idx(g1), "g2": _wrap_idx(g2),
            "dstv": np.ascontiguousarray(
                dv.reshape(ttot // 128, 128).T),
            "rs": rs, "rc": rc, "rs_par": rs_par, "rc_par": rc_par,
            "Wl1": np.asarray(inputs["Wl1"], np.float32),
            "Wr1": np.asarray(inputs["Wr1"], np.float32),
            "bl1": np.asarray(inputs["bl1"], np.float32).reshape(1, H),
            "Wl2": np.asarray(inputs["Wl2"], np.float32),
            "Wr2": np.asarray(inputs["Wr2"], np.float32),
            "bl2": np.asarray(inputs["bl2"], np.float32).reshape(1, D),
            "gamma": np.asarray(inputs["gamma"], np.float32).reshape(2 * D, 1),
            "beta": np.asarray(inputs["beta"], np.float32).reshape(2 * D, 1),
            "fc1w": np.asarray(inputs["fc1_w"], np.float32),
            "fc1b": np.asarray(inputs["fc1_b"], np.float32).reshape(512, 1),
            "fc2w": np.asarray(inputs["fc2_w"], np.float32),
            "fc2b": np.asarray(inputs["fc2_b"], np.float32).reshape(1, 2),
        })
    return budgets, blkrng, ttot, in_maps


def _build(budgets, blkrng, ttot, stage=99):
    nc = bacc.Bacc("TRN2", target_bir_lowering=False, debug=False,
                   num_devices=NCORES, num_swdge_queues=4,
                   dynamic_dma_scratch_size=32768)

    TBt = ttot // 128
    PTB = max(lo + hi for lo, hi in budgets) // 128   # max piece blocks
    NBM = max(le - ls + he - hs for ls, le, hs, he in blkrng)

    gab = nc.dram_tensor("gab", [N, D], BF16, kind="ExternalInput")
    eloc = nc.dram_tensor("eloc", [SH, D], F32, kind="ExternalInput")
    elocT = nc.dram_tensor("elocT", [D, SH], F32, kind="ExternalInput")
    g1 = nc.dram_tensor("g1", [128, ttot // 16], I16, kind="ExternalInput")
    g2 = nc.dram_tensor("g2", [128, ttot // 16], I16, kind="ExternalInput")
    dstv = nc.dram_tensor("dstv", [128, TBt], F32, kind="ExternalInput")
    rio = {k: nc.dram_tensor(k, [128, BSH * L // 16], I16, kind="ExternalInput")
           for k in ("rs", "rc")}
    rpar = {k: nc.dram_tensor(k, [128, (BSH // 128) * L], mybir.dt.int8,
                              kind="ExternalInput")
            for k in ("rs_par", "rc_par")}
    Wl1 = nc.dram_tensor("Wl1", [D, H], F32, kind="ExternalInput")
    Wr1 = nc.dram_tensor("Wr1", [D, H], F32, kind="ExternalInput")
    bl1 = nc.dram_tensor("bl1", [1, H], F32, kind="ExternalInput")
    Wl2 = nc.dram_tensor("Wl2", [H, D], F32, kind="ExternalInput")
    Wr2 = nc.dram_tensor("Wr2", [H, D], F32, kind="ExternalInput")
    bl2 = nc.dram_tensor("bl2", [1, D], F32, kind="ExternalInput")
    gamma = nc.dram_tensor("gamma", [2 * D, 1], F32, kind="ExternalInput")
    beta = nc.dram_tensor("beta", [2 * D, 1], F32, kind="ExternalInput")
    fc1w = nc.dram_tensor("fc1w", [2 * D, 512], F32, kind="ExternalInput")
    fc1b = nc.dram_tensor("fc1b", [512, 1], F32, kind="ExternalInput")
    fc2w = nc.dram_tensor("fc2w", [512, 2], F32, kind="ExternalInput")
    fc2b = nc.dram_tensor("fc2b", [1, 2], F32, kind="ExternalInput")
    cnt_in = nc.dram_tensor("cnt_in", [SH, 1], F32, kind="ExternalInput")
    out = nc.dram_tensor("out", [BSH, 2], F32, kind="ExternalOutput")

    poff = [0]
    for lo_b, hi_b in budgets:
        poff.append(poff[-1] + lo_b + hi_b)

    with tile.TileContext(nc) as tc:
        with tc.tile_pool(name="sb", bufs=1) as cpool, \
             tc.tile_pool(name="gt", bufs=2) as gpool, \
             tc.tile_pool(name="mm", bufs=3) as mpool, \
             tc.tile_pool(name="ps", bufs=2, space="PSUM") as ppool, \
             tc.tile_pool(name="ps1", bufs=1, space="PSUM") as qpool, \
             tc.tile_pool(name="dram", bufs=1, space="DRAM") as dpool:

            # ---- constants / index loads -------------------------------
            ident = cpool.tile([128, 128], F32)
            make_identity(nc, ident[:])
            ones = cpool.tile([1, 128], F32)
            nc.gpsimd.memset(ones[:], 1.0)
            iot32 = cpool.tile([128, 128], mybir.dt.int32)
            nc.gpsimd.iota(iot32[:], pattern=[[1, 128]], base=0,
                           channel_multiplier=0)
            iotf = cpool.tile([128, 128], F32)
            nc.vector.tensor_copy(iotf[:], iot32[:])

            rio_t = {}
            for k, d in rio.items():
                t = cpool.tile([128, BSH * L // 16], I16, tag=k, name=k)
                nc.sync.dma_start(t[:], d[:])
                rio_t[k] = t
            rpar_t = {}
            for k, d in rpar.items():
                t = cpool.tile([128, (BSH // 128) * L], mybir.dt.int8,
                               tag=k, name=k)
                nc.sync.dma_start(t[:], d[:])
                rpar_t[k] = t
            dstv_t = cpool.tile([128, TBt], F32)
            nc.sync.dma_start(dstv_t[:], dstv[:])

            wl1 = cpool.tile([D, H], F32)
            wr1 = cpool.tile([D, H], F32)
            b1 = cpool.tile([1, H], F32)
            # [256, D] weights packed K-chunk-major into 128 partitions
            wl2 = cpool.tile([128, 2 * D], F32)
            wr2 = cpool.tile([128, 2 * D], F32)
            b2 = cpool.tile([1, D], F32)
            nc.sync.dma_start(wl1[:], Wl1[:])
            nc.sync.dma_start(wr1[:], Wr1[:])
            nc.sync.dma_start(b1[:], bl1[:])
            for j in range(2):
                nc.sync.dma_start(wl2[:, j * D:(j + 1) * D],
                                  Wl2[j * 128:(j + 1) * 128, :])
                nc.sync.dma_start(wr2[:, j * D:(j + 1) * D],
                                  Wr2[j * 128:(j + 1) * 128, :])
            nc.sync.dma_start(b2[:], bl2[:])

            # DRAM bounce tensors for the collectives + x1T spill
            z_loc = dpool.tile([SHP, D], BF16)
            z_pad = dpool.tile([NP_, D], BF16)
            x_loc = dpool.tile([SHP, D], BF16)
            x_pad = dpool.tile([NP_, D], BF16)
            x1T_d = [dpool.tile([128, SH], F32, name=f"x1Td{j}")
                     for j in range(2)]
            zrowb = cpool.tile([1, D], BF16)
            nc.gpsimd.memset(zrowb[:], 0.0)
            nc.sync.dma_start(z_loc[SH:SH + 1, :], zrowb[:])
            nc.sync.dma_start(x_loc[SH:SH + 1, :], zrowb[:])

            rcnt_all = cpool.tile([128, NM], F32)

            # ---- piece gather + per-chunk segment-matmul helpers -------
            def issue_piece(p, gidx_d, tbl_lo, tbl_hi):
                lo_b, hi_b = budgets[p]
                tot = lo_b + hi_b
                gt = gpool.tile([128, PTB, D], BF16, tag="gt")
                gi = gpool.tile([128, PTB * 8], I16, tag="gi")
                nc.sync.dma_start(
                    gi[:, :tot // 16],
                    gidx_d[:, poff[p] // 16:(poff[p] + tot) // 16])
                if lo_b:
                    nc.gpsimd.dma_gather(
                        gt[:, :lo_b // 128, :], tbl_lo, gi[:, :lo_b // 16],
                        lo_b, lo_b, D, single_packet=False,
                        queue_num=(2 * p) % 4)
                if hi_b:
                    nc.gpsimd.dma_gather(
                        gt[:, lo_b // 128:tot // 128, :], tbl_hi,
                        gi[:, lo_b // 16:tot // 16], hi_b, hi_b, D,
                        single_packet=False, queue_num=(2 * p + 1) % 4)
                return gt

            def chunk_agg(m, gt):
                """Accumulate agg[dst,feat] for chunk m into a PSUM tile."""
                p = m // PCH
                ls, le, hs, he = blkrng[m]
                n1, n2 = le - ls, he - hs
                nb = n1 + n2
                blocks = list(range(ls, le)) + list(range(hs, he))
                r0 = m * 128
                mw = min(r0 + 128, SH) - r0
                gb0 = poff[p] // 128
                dsh = mpool.tile([128, NBM], F32, tag="dsh")
                if n1:
                    nc.vector.tensor_scalar_add(
                        dsh[:, :n1], dstv_t[:, gb0 + ls:gb0 + le],
                        -float(r0))
                if n2:
                    nc.vector.tensor_scalar_add(
                        dsh[:, n1:nb], dstv_t[:, gb0 + hs:gb0 + he],
                        -float(r0))
                oh = mpool.tile([128, NBM, 128], BF16, tag="oh")
                nc.vector.tensor_tensor(
                    oh[:, :nb, :],
                    dsh[:, :nb].unsqueeze(2).to_broadcast([128, nb, 128]),
                    iotf[:].unsqueeze(1).to_broadcast([128, nb, 128]),
                    mybir.AluOpType.is_equal)
                aggp = ppool.tile([128, D], F32, tag="aggp")
                for k, b in enumerate(blocks):
                    nc.tensor.matmul(aggp[:mw, :], oh[:, k, :mw],
                                     gt[:, b, :], start=(k == 0),
                                     stop=(k == len(blocks) - 1))
                return aggp, r0, mw

            import os
            stage = int(os.environ.get("KSTAGE", stage))

            # ---- conv1: per piece, gather + agg + dense ----------------
            gt_cur = issue_piece(0, g1, gab[:LOSPLIT], gab[LOSPLIT:])
            for p in range(NPC):
                gt_next = (issue_piece(p + 1, g1, gab[:LOSPLIT],
                                       gab[LOSPLIT:])
                           if p + 1 < NPC else None)
                for m in range(p * PCH, min((p + 1) * PCH, NM)):
                    aggp, r0, mw = chunk_agg(m, gt_cur)
                    r1 = r0 + mw
                    ct = mpool.tile([128, 1], F32, tag="ct")
                    nc.sync.dma_start(ct[:mw, :], cnt_in[r0:r1, :])
                    rc = rcnt_all[:, m:m + 1]
                    nc.vector.tensor_scalar_max(ct[:mw, :], ct[:mw, :], 1.0)
                    nc.vector.reciprocal(rc[:mw], ct[:mw, :])
                    mean = mpool.tile([128, D], F32, tag="mean")
                    nc.vector.tensor_scalar_mul(mean[:mw, :], aggp[:mw, :],
                                                rc[:mw])
                    mtp = ppool.tile([128, 128], F32, tag="tr")
                    nc.tensor.transpose(mtp[:, :mw], mean[:mw, :],
                                        ident[:mw, :mw])
                    meanT = mpool.tile([128, 128], F32, tag="meanT")
                    nc.vector.tensor_copy(meanT[:, :mw], mtp[:, :mw])
                    et = mpool.tile([128, 128], F32, tag="et")
                    nc.sync.dma_start(et[:, :mw], elocT[:, r0:r1])
                    ps = ppool.tile([128, H], F32, tag="mmps")
                    nc.tensor.matmul(ps[:mw, :], meanT[:, :mw], wl1[:],
                                     start=True, stop=False)
                    nc.tensor.matmul(ps[:mw, :], et[:, :mw], wr1[:],
                                     start=False, stop=False)
                    nc.tensor.matmul(ps[:mw, :], ones[:, :mw], b1[:],
                                     start=False, stop=True)
                    x1t = mpool.tile([128, H], F32, tag="x1t")
                    nc.scalar.activation(x1t[:mw, :], ps[:mw, :],
                                         mybir.ActivationFunctionType.Relu)
                    psz = qpool.tile([128, D], F32, tag="psz")
                    for j in range(2):
                        tp = ppool.tile([128, 128], F32, tag="tr")
                        nc.tensor.transpose(tp[:, :mw],
                                            x1t[:mw, j * 128:(j + 1) * 128],
                                            ident[:mw, :mw])
                        xts = mpool.tile([128, 128], F32, tag="xts")
                        nc.vector.tensor_copy(xts[:, :mw], tp[:, :mw])
                        nc.sync.dma_start(x1T_d[j][:, r0:r1], xts[:, :mw])
                        nc.tensor.matmul(psz[:mw, :], xts[:, :mw],
                                         wl2[:, j * D:(j + 1) * D],
                                         start=(j == 0), stop=(j == 1))
                    zt = mpool.tile([128, D], BF16, tag="zt")
                    nc.vector.tensor_copy(zt[:mw, :], psz[:mw, :])
                    nc.sync.dma_start(z_loc[r0:r1, :], zt[:mw, :])
                gt_cur = gt_next

            if stage < 3:
                return nc
            nc.gpsimd.collective_compute(
                "AllGather", mybir.AluOpType.bypass,
                replica_groups=[list(range(NCORES))],
                ins=[z_loc.opt()], outs=[z_pad.opt()])

            if stage < 4:
                return nc
            # ---- conv2: agg(z) + dense + residual ----------------------
            gt_cur = issue_piece(0, g2, z_pad[:PADLO], z_pad[PADLO:])
            for p in range(NPC):
                gt_next = (issue_piece(p + 1, g2, z_pad[:PADLO],
                                       z_pad[PADLO:])
                           if p + 1 < NPC else None)
                for m in range(p * PCH, min((p + 1) * PCH, NM)):
                    aggp, r0, mw = chunk_agg(m, gt_cur)
                    r1 = r0 + mw
                    m2 = mpool.tile([128, D], F32, tag="m2")
                    nc.vector.tensor_scalar_mul(m2[:mw, :], aggp[:mw, :],
                                                rcnt_all[:mw, m:m + 1])
                    ps = ppool.tile([128, D], F32, tag="mmps")
                    for j in range(2):
                        x1l = mpool.tile([128, 128], F32, tag="x1l")
                        nc.sync.dma_start(x1l[:, :mw], x1T_d[j][:, r0:r1])
                        nc.tensor.matmul(ps[:mw, :], x1l[:, :mw],
                                         wr2[:, j * D:(j + 1) * D],
                                         start=(j == 0), stop=False)
                    nc.tensor.matmul(ps[:mw, :], ones[:, :mw], b2[:],
                                     start=False, stop=True)
                    el = mpool.tile([128, D], F32, tag="el")
                    nc.sync.dma_start(el[:mw, :], eloc[r0:r1, :])
                    xt = mpool.tile([128, D], F32, tag="xt")
                    nc.vector.tensor_add(xt[:mw, :], ps[:mw, :], m2[:mw, :])
                    nc.vector.tensor_add(xt[:mw, :], xt[:mw, :], el[:mw, :])
                    xtb = mpool.tile([128, D], BF16, tag="xtb")
                    nc.vector.tensor_copy(xtb[:mw, :], xt[:mw, :])
                    nc.sync.dma_start(x_loc[r0:r1, :], xtb[:mw, :])
                gt_cur = gt_next

            if stage < 5:
                return nc
            nc.gpsimd.collective_compute(
                "AllGather", mybir.AluOpType.bypass,
                replica_groups=[list(range(NCORES))],
                ins=[x_loc.opt()], outs=[x_pad.opt()])

            if stage < 6:
                return nc
            # ---- readout: gather + strided L-reduction -> emdT ---------
            emdT = [cpool.tile([128, BSH], F32, tag=f"emdT{h}", name=f"emdT{h}")
                    for h in range(2)]
            nblk = BSH // 128
            x_packed = x_pad[:].rearrange("(a b) d -> a (b d)", b=2)
            LH = L // 2
            for h, (kidx, kpar) in enumerate((("rs", "rs_par"),
                                              ("rc", "rc_par"))):
                for blk in range(nblk):
                    red = [None, None]
                    for i in range(2):
                        c0 = (blk * 2 + i) * (LH * 128 // 16)
                        gt = gpool.tile([128, LH, 2 * D], BF16, tag="rgt")
                        nc.gpsimd.dma_gather(
                            gt[:], x_packed,
                            rio_t[kidx][:, c0:c0 + LH * 128 // 16],
                            LH * 128, LH * 128, 2 * D, single_packet=False,
                            queue_num=(2 * blk + i) % 4)
                        mk = rpar_t[kpar][:, (blk * 2 + i) * LH:
                                          (blk * 2 + i + 1) * LH]
                        nc.vector.copy_predicated(
                            gt[:, :, :D],
                            mk.unsqueeze(2).to_broadcast([128, LH, D]),
                            gt[:, :, D:])
                        rt = mpool.tile([128, D], F32, tag=f"red{i}")
                        nc.vector.tensor_reduce(
                            rt[:], gt[:, :, :D].rearrange("p l f -> p f l"),
                            mybir.AxisListType.X, mybir.AluOpType.add)
                        red[i] = rt
                    sb = mpool.tile([128, D], F32, tag="sb")
                    nc.vector.tensor_add(sb[:], red[0][:], red[1][:])
                    tp = ppool.tile([128, 128], F32, tag="tr")
                    nc.tensor.transpose(tp[:], sb[:], ident[:])
                    nc.vector.tensor_copy(
                        emdT[h][:, blk * 128:(blk + 1) * 128], tp[:])

            if stage < 7:
                return nc
            # ---- BatchNorm (batch stats across all cores) --------------
            stats_l = dpool.tile([128, 4], F32)
            stats_g = dpool.tile([128, 4], F32)
            st = cpool.tile([128, 4], F32)
            scratch = mpool.tile([128, BSH], F32, tag="scratch")
            for h in range(2):
                nc.vector.tensor_reduce(st[:, 2 * h:2 * h + 1], emdT[h][:],
                                        mybir.AxisListType.X,
                                        mybir.AluOpType.add)
                nc.scalar.activation(scratch[:], emdT[h][:],
                                     mybir.ActivationFunctionType.Square,
                                     accum_out=st[:, 2 * h + 1:2 * h + 2])
            nc.sync.dma_start(stats_l[:], st[:])
            nc.gpsimd.collective_compute(
                "AllReduce", mybir.AluOpType.add,
                replica_groups=[list(range(NCORES))],
                ins=[stats_l.opt()], outs=[stats_g.opt()])
            sg = cpool.tile([128, 4], F32)
            nc.sync.dma_start(sg[:], stats_g[:])
            gm = cpool.tile([128, 2], F32)
            bt = cpool.tile([128, 2], F32)
            for h in range(2):
                nc.sync.dma_start(gm[:, h:h + 1], gamma[h * 128:(h + 1) * 128, :])
                nc.sync.dma_start(bt[:, h:h + 1], beta[h * 128:(h + 1) * 128, :])
            for h in range(2):
                mu = cpool.tile([128, 1], F32, tag=f"mu{h}")
                var = cpool.tile([128, 1], F32, tag=f"var{h}")
                nc.scalar.mul(mu[:], sg[:, 2 * h:2 * h + 1], 1.0 / B)
                nc.scalar.mul(var[:], sg[:, 2 * h + 1:2 * h + 2], 1.0 / B)
                musq = cpool.tile([128, 1], F32, tag=f"musq{h}")
                nc.vector.tensor_mul(musq[:], mu[:], mu[:])
                nc.vector.tensor_sub(var[:], var[:], musq[:])
                nc.vector.tensor_scalar_add(var[:], var[:], EPS)
                nc.scalar.sqrt(var[:], var[:])
                rstd = cpool.tile([128, 1], F32, tag=f"rstd{h}")
                nc.vector.reciprocal(rstd[:], var[:])
                scale = cpool.tile([128, 1], F32, tag=f"scale{h}")
                nc.vector.tensor_mul(scale[:], gm[:, h:h + 1], rstd[:])
                shift = cpool.tile([128, 1], F32, tag=f"shift{h}")
                nc.vector.tensor_mul(shift[:], mu[:], scale[:])
                nc.vector.tensor_sub(shift[:], bt[:, h:h + 1], shift[:])
                nc.scalar.activation(emdT[h][:], emdT[h][:],
                                     mybir.ActivationFunctionType.Identity,
                                     bias=shift[:], scale=scale[:])

            # ---- MLP head ---------------------------------------------
            # fc1w [256,512] packed K-chunk-major: cols j*512..(j+1)*512
            f1w = cpool.tile([128, 1024], F32)
            for j in range(2):
                nc.sync.dma_start(f1w[:, j * 512:(j + 1) * 512],
                                  fc1w[j * 128:(j + 1) * 128, :])
            # fc2w [512,2] packed: cols 2k..2k+2 hold rows k*128..(k+1)*128
            f2w = cpool.tile([128, 8], F32)
            for k in range(4):
                nc.sync.dma_start(f2w[:, 2 * k:2 * k + 2],
                                  fc2w[k * 128:(k + 1) * 128, :])
            f2b = cpool.tile([1, 2], F32)
            nc.sync.dma_start(f2b[:], fc2b[:])
            h1T = []
            for k in range(4):
                ps = ppool.tile([128, BSH], F32, tag="mmps")
                for j in range(2):
                    nc.tensor.matmul(ps[:], f1w[:, j * 512 + k * 128:
                                                j * 512 + (k + 1) * 128],
                                     emdT[j][:], start=(j == 0), stop=(j == 1))
                f1b = cpool.tile([128, 1], F32, tag=f"f1b{k}")
                nc.sync.dma_start(f1b[:], fc1b[k * 128:(k + 1) * 128, :])
                ht = cpool.tile([128, BSH], F32, tag=f"h1T{k}")
                nc.scalar.activation(ht[:], ps[:],
                                     mybir.ActivationFunctionType.Relu,
                                     bias=f1b[:])
                h1T.append(ht)
            ot = mpool.tile([128, 2], F32, tag="ot")
            for m in range(4):
                ps = qpool.tile([128, 2], F32, tag="psz")
                for k in range(4):
                    nc.tensor.matmul(ps[:], h1T[k][:, m * 128:(m + 1) * 128],
                                     f2w[:, 2 * k:2 * k + 2],
                                     start=(k == 0), stop=False)
                nc.tensor.matmul(ps[:], ones[:], f2b[:], start=False, stop=True)
                nc.vector.tensor_copy(ot[:], ps[:])
                nc.sync.dma_start(out[m * 128:(m + 1) * 128, :], ot[:])
    return nc


def kernel(**inputs) -> np.ndarray:
    if "nc" not in _cache:
        budgets, blkrng, ttot, in_maps = _prepare(inputs)
        nc = _build(budgets, blkrng, ttot)
        nc.compile()
        _cache.update(nc=nc, in_maps=in_maps)
    res = run_bass_kernel_spmd(_cache["nc"], _cache["in_maps"],
                               list(range(NCORES)))
    _cache["last_results"] = res
    return np.concatenate([res.results[c]["out"] for c in range(NCORES)], 0)


# revision 5
# speedup vs baseline: 1.2017x; 1.2017x over previous
"""GCNContext GNN kernel for 8 TRN2 NeuronCores (Bass/Tile, SPMD).

Reference computation (see harness):
    x1 = relu(SAGE(emb; Wl1,bl1,Wr1));  x2 = SAGE(x1; Wl2,bl2,Wr2)
    x  = x2 + emb
    emd = [sum_l x[sentence], sum_l x[context]]  -> BatchNorm -> MLP -> [B,2]

Distribution: nodes+edges sharded by dst core (6250/core), MLP head
replicated, batch rows data-parallel (512/core).

v2 design (segment-matmul aggregation — no DMA scatter):
  * segment-sum of x[src] over dst: GPSIMD dma_gather pulls edge src rows
    (bf16, 256B packets) into SBUF in dst-chunk-grouped order; per
    128-token block a one-hot matrix (DVE iota+is_equal, bf16) selects
    each token's dst row, and PE matmuls accumulate agg[dst(128), feat]
    in PSUM across the chunk's blocks. Replaces the v1 dma_scatter_add
    chains (1.5ms serial GPSIMD desc-gen + 200k RMW descriptors) and the
    bf16->f32 CAST of the gathered stream.
  * conv2 pre-multiply: z = x1 @ Wl2 is computed in the conv1 dense loop
    (reusing the x1^T transposes), AllGathered bf16 [50008,128], and
    aggregated instead of x1 — mean2 @ Wl2 == (Adj z)/cnt. Halves conv2
    gather bytes and the inter-conv collective vs gathering x1 (256-wide).
  * edges are grouped into pieces of 8 dst chunks; each piece issues two
    gathers (lo/hi int16 table halves) on rotating SWDGE queues (4) with
    pool bufs=2 double buffering, so desc-gen/DMA of piece p+1 overlap
    PE/DVE of piece p. Chunk boundaries inside a piece are not
    128-aligned; boundary blocks are matmul'd into both chunks (out-of-
    window dst ids fail the base-shifted is_equal, adding zero).
  * in-degree counts from host metadata; readout via pair-packed bf16
    x_pad view + parity copy_predicated + strided L-reduction (as v1);
    BatchNorm stats AllReduced; MLP replicated per 512-row batch shard.

Perf history (HW exec, NTFF): 7.74ms f32 scatter baseline -> ... ->
5.33ms v1 best (bf16 tables, 2 SWDGE queues, pair-packed readout).
"""
import sys

sys.path.insert(0, "/opt/trn_rl_repo")

import numpy as np

import concourse.bacc as bacc
import concourse.bass as bass
import concourse.mybir as mybir
import concourse.tile as tile
from concourse.bass_utils import run_bass_kernel_spmd
from concourse.masks import make_identity

NCORES = 8
N, D, H, B, L = 50000, 128, 256, 4096, 50
SH = N // NCORES          # 6250 nodes per shard
BSH = B // NCORES         # 512 batch rows per core
LOSPLIT = 25000           # node-id split for int16 gather tables (conv1)
SHP = SH + 1              # padded shard rows (zero row at 6250)
NP_ = NCORES * SHP        # 50008 padded table rows
PADLO = (NCORES // 2) * SHP   # 25004: row split of the padded tables
NM = (SH + 127) // 128    # 49 dst chunks per core (last has 106 rows)
PCH = 8                   # dst chunks per gather piece
NPC = (NM + PCH - 1) // PCH
EPS = 1e-5
F32 = mybir.dt.float32
BF16 = mybir.dt.bfloat16
I16 = mybir.dt.int16

_cache = {}


def _wrap_idx(a):
    """1-D int array (len % 16 == 0) -> [128, n/16] int16 wrapped layout."""
    a16 = np.asarray(a, np.int64).reshape(-1, 16).T.astype(np.int16)
    return np.tile(a16, (8, 1))


def _padmap(n):
    """node id -> row in the padded (zero-row-per-shard) tables."""
    return (n // SH) * SHP + (n % SH)


def _ceil128(x):
    return (int(x) + 127) // 128 * 128


def _plan_edges(src, dst):
    """Group edges per core into (piece, lo/hi-half) streams.

    Streams are ordered by (dst chunk, src) inside each half; budgets
    (max over cores, ceil128) and per-chunk block ranges are shared by
    the SPMD program; per-core shortfall is padded with dst=-1 tokens.

    Returns (budgets, blkrng, percore):
      budgets[p] = (lo_b, hi_b) tokens, each a multiple of 128
      blkrng[m] = (ls, le, hs, he) block ranges in piece m//PCH's tile
      percore[c][p][h] = (src_ids, dst_local) sorted by (chunk, src)
    """
    core = dst // SH
    percore = []
    # cum[c][p][h]: per-chunk token start offsets within the stream
    starts = np.zeros((NCORES, NPC, 2, PCH + 1), np.int64)
    for c in range(NCORES):
        msk = core == c
        s_c, ld = src[msk], dst[msk] - c * SH
        ch = ld // 128
        pc = ch // PCH
        lo = s_c < LOSPLIT
        plist = []
        for p in range(NPC):
            halves = []
            for h, hm in enumerate((lo, ~lo)):
                sel = (pc == p) & hm
                ss, dd, cc = s_c[sel], ld[sel], ch[sel]
                o = np.lexsort((ss, cc))
                ss, dd, cc = ss[o], dd[o], cc[o]
                halves.append((ss, dd))
                for j, m in enumerate(range(p * PCH,
                                            min((p + 1) * PCH, NM))):
                    starts[c, p, h, j + 1] = starts[c, p, h, j] + int(
                        (cc == m).sum())
                starts[c, p, h, PCH] = len(ss)
            plist.append(halves)
        percore.append(plist)

    budgets = []
    for p in range(NPC):
        lo_b = _ceil128(max(len(percore[c][p][0][0]) for c in range(NCORES)))
        hi_b = _ceil128(max(len(percore[c][p][1][0]) for c in range(NCORES)))
        budgets.append((lo_b, hi_b))

    blkrng = []
    for m in range(NM):
        p, j = m // PCH, m % PCH
        lo_b = budgets[p][0]
        ls = int(starts[:, p, 0, j].min()) // 128
        le = -(-int(starts[:, p, 0, j + 1].max()) // 128)
        hs = lo_b // 128 + int(starts[:, p, 1, j].min()) // 128
        he = lo_b // 128 - (-int(starts[:, p, 1, j + 1].max()) // 128)
        # clip to the piece (ceil128 budget may exceed last chunk's end)
        le = min(le, lo_b // 128)
        he = min(he, (lo_b + budgets[p][1]) // 128)
        assert ls < le or hs < he, f"empty chunk {m}"
        blkrng.append((ls, le, hs, he))
    return budgets, blkrng, percore


def _readout_idx(tok):
    """[BSH, L] padded-table row ids -> pair-packed idx + parity mask."""
    nblk = BSH // 128
    m = tok.reshape(nblk, 128, L).transpose(0, 2, 1)       # [blk, l, p]
    m = m.reshape(nblk, 2, L // 2, 128)                    # [blk, h, lp, p]
    idx = (m // 2).reshape(-1)
    par = (m % 2).astype(np.int8)
    par_t = np.ascontiguousarray(
        par.transpose(3, 0, 1, 2).reshape(128, nblk * L))  # [p, blk*50+h*25+lp]
    return _wrap_idx(idx), par_t


def _prepare(inputs):
    src = np.asarray(inputs["edge_index"][0], np.int64)
    dst = np.asarray(inputs["edge_index"][1], np.int64)
    emb = np.asarray(inputs["emb"], np.float32)

    budgets, blkrng, percore = _plan_edges(src, dst)
    ttot = sum(lo + hi for lo, hi in budgets)

    import ml_dtypes
    gab = emb.astype(ml_dtypes.bfloat16)

    sent = np.asarray(inputs["sentence"], np.int64)
    cont = np.asarray(inputs["context"], np.int64)
    core_arr = dst // SH

    in_maps = []
    for c in range(NCORES):
        g1 = np.zeros(ttot, np.int64)
        g2 = np.zeros(ttot, np.int64)
        dv = np.full(ttot, -1.0, np.float32)
        pos = 0
        for p in range(NPC):
            for h in range(2):
                ss, dd = percore[c][p][h]
                n = len(ss)
                if h == 0:
                    g1[pos:pos + n] = ss
                    g2[pos:pos + n] = _padmap(ss)
                else:
                    g1[pos:pos + n] = ss - LOSPLIT
                    g2[pos:pos + n] = _padmap(ss) - PADLO
                dv[pos:pos + n] = dd
                pos += budgets[p][h]
        assert pos == ttot

        rs, rs_par = _readout_idx(_padmap(sent[c * BSH:(c + 1) * BSH]))
        rc, rc_par = _readout_idx(_padmap(cont[c * BSH:(c + 1) * BSH]))

        deg = np.bincount(dst[core_arr == c] - c * SH,
                          minlength=SH).astype(np.float32)
        degp = np.full(NM * 128, 1.0, np.float32)
        degp[:SH] = deg
        sl = slice(c * SH, (c + 1) * SH)
        in_maps.append({
            "cnt_in": np.ascontiguousarray(degp.reshape(NM, 128).T),
            "gab": gab,
            "eloc": emb[sl].copy(),
            "elocT": np.ascontiguousarray(emb[sl].T),
            "g1": _wrap_idx(g1), "g2": _wrap_idx(g2),
            "dstv": np.ascontiguousarray(
                dv.reshape(ttot // 128, 128).T),
            "rs": rs, "rc": rc, "rs_par": rs_par, "rc_par": rc_par,
            "Wl1": np.asarray(inputs["Wl1"], np.float32),
            "Wr1": np.asarray(inputs["Wr1"], np.float32),
            "bl1": np.asarray(inputs["bl1"], np.float32).reshape(1, H),
            "Wl2": np.asarray(inputs["Wl2"], np.float32),
            "Wr2": np.asarray(inputs["Wr2"], np.float32),
            "bl2": np.asarray(inputs["bl2"], np.float32).reshape(1, D),
            "gamma": np.asarray(inputs["gamma"], np.float32).reshape(2 * D, 1),
            "beta": np.asarray(inputs["beta"], np.float32).reshape(2 * D, 1),
            "fc1w": np.asarray(inputs["fc1_w"], np.float32),
            "fc1b": np.asarray(inputs["fc1_b"], np.float32).reshape(512, 1),
            "fc2w": np.asarray(inputs["fc2_w"], np.float32),
            "fc2b": np.asarray(inputs["fc2_b"], np.float32).reshape(1, 2),
        })
    return budgets, blkrng, ttot, in_maps


def _build(budgets, blkrng, ttot, stage=99):
    nc = bacc.Bacc("TRN2", target_bir_lowering=False, debug=False,
                   num_devices=NCORES, num_swdge_queues=4,
                   dynamic_dma_scratch_size=32768)

    TBt = ttot // 128
    PTB = max(lo + hi for lo, hi in budgets) // 128   # max piece blocks
    NBM = max(le - ls + he - hs for ls, le, hs, he in blkrng)

    gab = nc.dram_tensor("gab", [N, D], BF16, kind="ExternalInput")
    eloc = nc.dram_tensor("eloc", [SH, D], F32, kind="ExternalInput")
    elocT = nc.dram_tensor("elocT", [D, SH], F32, kind="ExternalInput")
    g1 = nc.dram_tensor("g1", [128, ttot // 16], I16, kind="ExternalInput")
    g2 = nc.dram_tensor("g2", [128, ttot // 16], I16, kind="ExternalInput")
    dstv = nc.dram_tensor("dstv", [128, TBt], F32, kind="ExternalInput")
    rio = {k: nc.dram_tensor(k, [128, BSH * L // 16], I16, kind="ExternalInput")
           for k in ("rs", "rc")}
    rpar = {k: nc.dram_tensor(k, [128, (BSH // 128) * L], mybir.dt.int8,
                              kind="ExternalInput")
            for k in ("rs_par", "rc_par")}
    Wl1 = nc.dram_tensor("Wl1", [D, H], F32, kind="ExternalInput")
    Wr1 = nc.dram_tensor("Wr1", [D, H], F32, kind="ExternalInput")
    bl1 = nc.dram_tensor("bl1", [1, H], F32, kind="ExternalInput")
    Wl2 = nc.dram_tensor("Wl2", [H, D], F32, kind="ExternalInput")
    Wr2 = nc.dram_tensor("Wr2", [H, D], F32, kind="ExternalInput")
    bl2 = nc.dram_tensor("bl2", [1, D], F32, kind="ExternalInput")
    gamma = nc.dram_tensor("gamma", [2 * D, 1], F32, kind="ExternalInput")
    beta = nc.dram_tensor("beta", [2 * D, 1], F32, kind="ExternalInput")
    fc1w = nc.dram_tensor("fc1w", [2 * D, 512], F32, kind="ExternalInput")
    fc1b = nc.dram_tensor("fc1b", [512, 1], F32, kind="ExternalInput")
    fc2w = nc.dram_tensor("fc2w", [512, 2], F32, kind="ExternalInput")
    fc2b = nc.dram_tensor("fc2b", [1, 2], F32, kind="ExternalInput")
    cnt_in = nc.dram_tensor("cnt_in", [128, NM], F32, kind="ExternalInput")
    out = nc.dram_tensor("out", [BSH, 2], F32, kind="ExternalOutput")

    poff = [0]
    for lo_b, hi_b in budgets:
        poff.append(poff[-1] + lo_b + hi_b)

    with tile.TileContext(nc) as tc:
        with tc.tile_pool(name="sb", bufs=1) as cpool, \
             tc.tile_pool(name="gt", bufs=2) as gpool, \
             tc.tile_pool(name="mm", bufs=3) as mpool, \
             tc.tile_pool(name="ps", bufs=2, space="PSUM") as ppool, \
             tc.tile_pool(name="dram", bufs=1, space="DRAM") as dpool:

            # ---- constants / index loads -------------------------------
            ident = cpool.tile([128, 128], F32)
            make_identity(nc, ident[:])
            ones = cpool.tile([1, 128], F32)
            nc.gpsimd.memset(ones[:], 1.0)
            iot32 = cpool.tile([128, 128], mybir.dt.int32)
            nc.gpsimd.iota(iot32[:], pattern=[[1, 128]], base=0,
                           channel_multiplier=0)
            iotf = cpool.tile([128, 128], F32)
            nc.vector.tensor_copy(iotf[:], iot32[:])

            rio_t = {}
            for k, d in rio.items():
                t = cpool.tile([128, BSH * L // 16], I16, tag=k, name=k)
                nc.sync.dma_start(t[:], d[:])
                rio_t[k] = t
            rpar_t = {}
            for k, d in rpar.items():
                t = cpool.tile([128, (BSH // 128) * L], mybir.dt.int8,
                               tag=k, name=k)
                nc.sync.dma_start(t[:], d[:])
                rpar_t[k] = t
            dstv_t = cpool.tile([128, TBt], F32)
            nc.sync.dma_start(dstv_t[:], dstv[:])

            wl1 = cpool.tile([D, H], F32)
            wr1 = cpool.tile([D, H], F32)
            b1 = cpool.tile([1, H], F32)
            # [256, D] weights packed K-chunk-major into 128 partitions
            wl2 = cpool.tile([128, 2 * D], F32)
            wr2 = cpool.tile([128, 2 * D], F32)
            b2 = cpool.tile([1, D], F32)
            nc.sync.dma_start(wl1[:], Wl1[:])
            nc.sync.dma_start(wr1[:], Wr1[:])
            nc.sync.dma_start(b1[:], bl1[:])
            for j in range(2):
                nc.sync.dma_start(wl2[:, j * D:(j + 1) * D],
                                  Wl2[j * 128:(j + 1) * 128, :])
                nc.sync.dma_start(wr2[:, j * D:(j + 1) * D],
                                  Wr2[j * 128:(j + 1) * 128, :])
            nc.sync.dma_start(b2[:], bl2[:])

            # DRAM bounce tensors for the collectives + x1T spill
            z_loc = dpool.tile([SHP, D], BF16)
            z_pad = dpool.tile([NP_, D], BF16)
            x_loc = dpool.tile([SHP, D], BF16)
            x_pad = dpool.tile([NP_, D], BF16)
            x1T_d = [dpool.tile([128, SH], F32, name=f"x1Td{j}")
                     for j in range(2)]
            zrowb = cpool.tile([1, D], BF16)
            nc.gpsimd.memset(zrowb[:], 0.0)
            nc.sync.dma_start(z_loc[SH:SH + 1, :], zrowb[:])
            nc.sync.dma_start(x_loc[SH:SH + 1, :], zrowb[:])

            rcnt_all = cpool.tile([128, NM], F32)
            cntw = cpool.tile([128, NM], F32)
            nc.sync.dma_start(cntw[:], cnt_in[:])
            nc.vector.tensor_scalar_max(cntw[:], cntw[:], 1.0)
            nc.vector.reciprocal(rcnt_all[:], cntw[:])

            # ---- piece gather + per-chunk segment-matmul helpers -------
            def issue_piece(p, gidx_d, tbl_lo, tbl_hi):
                lo_b, hi_b = budgets[p]
                tot = lo_b + hi_b
                gt = gpool.tile([128, PTB, D], BF16, tag="gt")
                gi = gpool.tile([128, PTB * 8], I16, tag="gi")
                nc.sync.dma_start(
                    gi[:, :tot // 16],
                    gidx_d[:, poff[p] // 16:(poff[p] + tot) // 16])
                if lo_b:
                    nc.gpsimd.dma_gather(
                        gt[:, :lo_b // 128, :], tbl_lo, gi[:, :lo_b // 16],
                        lo_b, lo_b, D, single_packet=False,
                        queue_num=(2 * p) % 4)
                if hi_b:
                    nc.gpsimd.dma_gather(
                        gt[:, lo_b // 128:tot // 128, :], tbl_hi,
                        gi[:, lo_b // 16:tot // 16], hi_b, hi_b, D,
                        single_packet=False, queue_num=(2 * p + 1) % 4)
                return gt

            def chunk_agg(m, gt):
                """Accumulate agg[dst,feat] for chunk m into a PSUM tile."""
                p = m // PCH
                ls, le, hs, he = blkrng[m]
                n1, n2 = le - ls, he - hs
                nb = n1 + n2
                blocks = list(range(ls, le)) + list(range(hs, he))
                r0 = m * 128
                mw = min(r0 + 128, SH) - r0
                gb0 = poff[p] // 128
                dsh = mpool.tile([128, NBM], F32, tag="dsh")
                if n1:
                    nc.vector.tensor_scalar_add(
                        dsh[:, :n1], dstv_t[:, gb0 + ls:gb0 + le],
                        -float(r0))
                if n2:
                    nc.vector.tensor_scalar_add(
                        dsh[:, n1:nb], dstv_t[:, gb0 + hs:gb0 + he],
                        -float(r0))
                oh = mpool.tile([128, NBM, 128], BF16, tag="oh")
                nc.vector.tensor_tensor(
                    oh[:, :nb, :],
                    dsh[:, :nb].unsqueeze(2).to_broadcast([128, nb, 128]),
                    iotf[:].unsqueeze(1).to_broadcast([128, nb, 128]),
                    mybir.AluOpType.is_equal)
                aggp = ppool.tile([128, D], F32, tag="aggp")
                for k, b in enumerate(blocks):
                    nc.tensor.matmul(aggp[:mw, :], oh[:, k, :mw],
                                     gt[:, b, :], start=(k == 0),
                                     stop=(k == len(blocks) - 1))
                return aggp, r0, mw

            import os
            stage = int(os.environ.get("KSTAGE", stage))

            # ---- conv1: per piece, gather + agg + dense (staggered) ----
            def conv1_dense(aggp, m):
                r0 = m * 128
                mw = min(r0 + 128, SH) - r0
                r1 = r0 + mw
                mean = mpool.tile([128, D], F32, tag="mean")
                nc.vector.tensor_scalar_mul(mean[:mw, :], aggp[:mw, :],
                                            rcnt_all[:mw, m:m + 1])
                mtp = ppool.tile([128, 128], F32, tag="tr")
                nc.tensor.transpose(mtp[:, :mw], mean[:mw, :],
                                    ident[:mw, :mw])
                meanT = mpool.tile([128, 128], F32, tag="meanT")
                nc.scalar.activation(meanT[:, :mw], mtp[:, :mw],
                                     mybir.ActivationFunctionType.Identity)
                et = mpool.tile([128, 128], F32, tag="et")
                nc.sync.dma_start(et[:, :mw], elocT[:, r0:r1])
                ps = ppool.tile([128, H], F32, tag="mmps")
                nc.tensor.matmul(ps[:mw, :], meanT[:, :mw], wl1[:],
                                 start=True, stop=False)
                nc.tensor.matmul(ps[:mw, :], et[:, :mw], wr1[:],
                                 start=False, stop=False)
                nc.tensor.matmul(ps[:mw, :], ones[:, :mw], b1[:],
                                 start=False, stop=True)
                x1t = mpool.tile([128, H], F32, tag="x1t")
                nc.scalar.activation(x1t[:mw, :], ps[:mw, :],
                                     mybir.ActivationFunctionType.Relu)
                psz = ppool.tile([128, D], F32, tag="psz")
                for j in range(2):
                    tp = ppool.tile([128, 128], F32, tag="tr")
                    nc.tensor.transpose(tp[:, :mw],
                                        x1t[:mw, j * 128:(j + 1) * 128],
                                        ident[:mw, :mw])
                    xts = mpool.tile([128, 128], F32, tag="xts")
                    nc.scalar.activation(xts[:, :mw], tp[:, :mw],
                                         mybir.ActivationFunctionType.Identity)
                    nc.sync.dma_start(x1T_d[j][:, r0:r1], xts[:, :mw])
                    nc.tensor.matmul(psz[:mw, :], xts[:, :mw],
                                     wl2[:, j * D:(j + 1) * D],
                                     start=(j == 0), stop=(j == 1))
                zt = mpool.tile([128, D], BF16, tag="zt")
                nc.vector.tensor_copy(zt[:mw, :], psz[:mw, :])
                nc.sync.dma_start(z_loc[r0:r1, :], zt[:mw, :])

            gt_cur = issue_piece(0, g1, gab[:LOSPLIT], gab[LOSPLIT:])
            pend = None
            for p in range(NPC):
                gt_next = (issue_piece(p + 1, g1, gab[:LOSPLIT],
                                       gab[LOSPLIT:])
                           if p + 1 < NPC else None)
                for m in range(p * PCH, min((p + 1) * PCH, NM)):
                    aggp, r0, mw = chunk_agg(m, gt_cur)
                    if pend is not None:
                        conv1_dense(*pend)
                    pend = (aggp, m)
                gt_cur = gt_next
            conv1_dense(*pend)

            if stage < 3:
                return nc
            nc.gpsimd.collective_compute(
                "AllGather", mybir.AluOpType.bypass,
                replica_groups=[list(range(NCORES))],
                ins=[z_loc.opt()], outs=[z_pad.opt()])

            if stage < 4:
                return nc
            # ---- conv2: agg(z) + dense + residual (staggered) ----------
            def conv2_dense(aggp, m):
                r0 = m * 128
                mw = min(r0 + 128, SH) - r0
                r1 = r0 + mw
                m2 = mpool.tile([128, D], F32, tag="m2")
                nc.vector.tensor_scalar_mul(m2[:mw, :], aggp[:mw, :],
                                            rcnt_all[:mw, m:m + 1])
                ps = ppool.tile([128, D], F32, tag="mmps")
                for j in range(2):
                    x1l = mpool.tile([128, 128], F32, tag="x1l")
                    nc.sync.dma_start(x1l[:, :mw], x1T_d[j][:, r0:r1])
                    nc.tensor.matmul(ps[:mw, :], x1l[:, :mw],
                                     wr2[:, j * D:(j + 1) * D],
                                     start=(j == 0), stop=False)
                nc.tensor.matmul(ps[:mw, :], ones[:, :mw], b2[:],
                                 start=False, stop=True)
                el = mpool.tile([128, D], F32, tag="el")
                nc.sync.dma_start(el[:mw, :], eloc[r0:r1, :])
                xt = mpool.tile([128, D], F32, tag="xt")
                nc.vector.tensor_add(xt[:mw, :], ps[:mw, :], m2[:mw, :])
                nc.vector.tensor_add(xt[:mw, :], xt[:mw, :], el[:mw, :])
                xtb = mpool.tile([128, D], BF16, tag="xtb")
                nc.scalar.activation(xtb[:mw, :], xt[:mw, :],
                                     mybir.ActivationFunctionType.Identity)
                nc.sync.dma_start(x_loc[r0:r1, :], xtb[:mw, :])

            gt_cur = issue_piece(0, g2, z_pad[:PADLO], z_pad[PADLO:])
            pend = None
            for p in range(NPC):
                gt_next = (issue_piece(p + 1, g2, z_pad[:PADLO],
                                       z_pad[PADLO:])
                           if p + 1 < NPC else None)
                for m in range(p * PCH, min((p + 1) * PCH, NM)):
                    aggp, r0, mw = chunk_agg(m, gt_cur)
                    if pend is not None:
                        conv2_dense(*pend)
                    pend = (aggp, m)
                gt_cur = gt_next
            conv2_dense(*pend)

            if stage < 5:
                return nc
            nc.gpsimd.collective_compute(
                "AllGather", mybir.AluOpType.bypass,
                replica_groups=[list(range(NCORES))],
                ins=[x_loc.opt()], outs=[x_pad.opt()])

            if stage < 6:
                return nc
            # ---- readout: gather + strided L-reduction -> emdT ---------
            emdT = [cpool.tile([128, BSH], F32, tag=f"emdT{h}", name=f"emdT{h}")
                    for h in range(2)]
            nblk = BSH // 128
            x_packed = x_pad[:].rearrange("(a b) d -> a (b d)", b=2)
            LH = L // 2
            for h, (kidx, kpar) in enumerate((("rs", "rs_par"),
                                              ("rc", "rc_par"))):
                for blk in range(nblk):
                    red = [None, None]
                    for i in range(2):
                        c0 = (blk * 2 + i) * (LH * 128 // 16)
                        gt = gpool.tile([128, LH, 2 * D], BF16, tag="rgt")
                        nc.gpsimd.dma_gather(
                            gt[:], x_packed,
                            rio_t[kidx][:, c0:c0 + LH * 128 // 16],
                            LH * 128, LH * 128, 2 * D, single_packet=False,
                            queue_num=(2 * blk + i) % 4)
                        mk = rpar_t[kpar][:, (blk * 2 + i) * LH:
                                          (blk * 2 + i + 1) * LH]
                        nc.vector.copy_predicated(
                            gt[:, :, :D],
                            mk.unsqueeze(2).to_broadcast([128, LH, D]),
                            gt[:, :, D:])
                        rt = mpool.tile([128, D], F32, tag=f"red{i}")
                        nc.vector.tensor_reduce(
                            rt[:], gt[:, :, :D].rearrange("p l f -> p f l"),
                            mybir.AxisListType.X, mybir.AluOpType.add)
                        red[i] = rt
                    sb = mpool.tile([128, D], F32, tag="sb")
                    nc.vector.tensor_add(sb[:], red[0][:], red[1][:])
                    tp = ppool.tile([128, 128], F32, tag="tr")
                    nc.tensor.transpose(tp[:], sb[:], ident[:])
                    nc.vector.tensor_copy(
                        emdT[h][:, blk * 128:(blk + 1) * 128], tp[:])

            if stage < 7:
                return nc
            # ---- BatchNorm (batch stats across all cores) --------------
            stats_l = dpool.tile([128, 4], F32)
            stats_g = dpool.tile([128, 4], F32)
            st = cpool.tile([128, 4], F32)
            scratch = mpool.tile([128, BSH], F32, tag="scratch")
            for h in range(2):
                nc.vector.tensor_reduce(st[:, 2 * h:2 * h + 1], emdT[h][:],
                                        mybir.AxisListType.X,
                                        mybir.AluOpType.add)
                nc.scalar.activation(scratch[:], emdT[h][:],
                                     mybir.ActivationFunctionType.Square,
                                     accum_out=st[:, 2 * h + 1:2 * h + 2])
            nc.sync.dma_start(stats_l[:], st[:])
            nc.gpsimd.collective_compute(
                "AllReduce", mybir.AluOpType.add,
                replica_groups=[list(range(NCORES))],
                ins=[stats_l.opt()], outs=[stats_g.opt()])
            sg = cpool.tile([128, 4], F32)
            nc.sync.dma_start(sg[:], stats_g[:])
            gm = cpool.tile([128, 2], F32)
            bt = cpool.tile([128, 2], F32)
            for h in range(2):
                nc.sync.dma_start(gm[:, h:h + 1], gamma[h * 128:(h + 1) * 128, :])
                nc.sync.dma_start(bt[:, h:h + 1], beta[h * 128:(h + 1) * 128, :])
            for h in range(2):
                mu = cpool.tile([128, 1], F32, tag=f"mu{h}")
                var = cpool.tile([128, 1], F32, tag=f"var{h}")
                nc.scalar.mul(mu[:], sg[:, 2 * h:2 * h + 1], 1.0 / B)
                nc.scalar.mul(var[:], sg[:, 2 * h + 1:2 * h + 2], 1.0 / B)
                musq = cpool.tile([128, 1], F32, tag=f"musq{h}")
                nc.vector.tensor_mul(musq[:], mu[:], mu[:])
                nc.vector.tensor_sub(var[:], var[:], musq[:])
                nc.vector.tensor_scalar_add(var[:], var[:], EPS)
                nc.scalar.sqrt(var[:], var[:])
                rstd = cpool.tile([128, 1], F32, tag=f"rstd{h}")
                nc.vector.reciprocal(rstd[:], var[:])
                scale = cpool.tile([128, 1], F32, tag=f"scale{h}")
                nc.vector.tensor_mul(scale[:], gm[:, h:h + 1], rstd[:])
                shift = cpool.tile([128, 1], F32, tag=f"shift{h}")
                nc.vector.tensor_mul(shift[:], mu[:], scale[:])
                nc.vector.tensor_sub(shift[:], bt[:, h:h + 1], shift[:])
                nc.scalar.activation(emdT[h][:], emdT[h][:],
                                     mybir.ActivationFunctionType.Identity,
                                     bias=shift[:], scale=scale[:])

            # ---- MLP head ---------------------------------------------
            # fc1w [256,512] packed K-chunk-major: cols j*512..(j+1)*512
            f1w = cpool.tile([128, 1024], F32)
            for j in range(2):
                nc.sync.dma_start(f1w[:, j * 512:(j + 1) * 512],
                                  fc1w[j * 128:(j + 1) * 128, :])
            # fc2w [512,2] packed: cols 2k..2k+2 hold rows k*128..(k+1)*128
            f2w = cpool.tile([128, 8], F32)
            for k in range(4):
                nc.sync.dma_start(f2w[:, 2 * k:2 * k + 2],
                                  fc2w[k * 128:(k + 1) * 128, :])
            f2b = cpool.tile([1, 2], F32)
            nc.sync.dma_start(f2b[:], fc2b[:])
            h1T = []
            for k in range(4):
                ps = ppool.tile([128, BSH], F32, tag="mmps")
                for j in range(2):
                    nc.tensor.matmul(ps[:], f1w[:, j * 512 + k * 128:
                                                j * 512 + (k + 1) * 128],
                                     emdT[j][:], start=(j == 0), stop=(j == 1))
                f1b = cpool.tile([128, 1], F32, tag=f"f1b{k}")
                nc.sync.dma_start(f1b[:], fc1b[k * 128:(k + 1) * 128, :])
                ht = cpool.tile([128, BSH], F32, tag=f"h1T{k}")
                nc.scalar.activation(ht[:], ps[:],
                                     mybir.ActivationFunctionType.Relu,
                                     bias=f1b[:])
                h1T.append(ht)
            ot = mpool.tile([128, 2], F32, tag="ot")
            for m in range(4):
                ps = ppool.tile([128, 2], F32, tag="psz")
                for k in range(4):
                    nc.tensor.matmul(ps[:], h1T[k][:, m * 128:(m + 1) * 128],
                                     f2w[:, 2 * k:2 * k + 2],
                                     start=(k == 0), stop=False)
                nc.tensor.matmul(ps[:], ones[:], f2b[:], start=False, stop=True)
                nc.vector.tensor_copy(ot[:], ps[:])
                nc.sync.dma_start(out[m * 128:(m + 1) * 128, :], ot[:])
    return nc


def kernel(**inputs) -> np.ndarray:
    if "nc" not in _cache:
        budgets, blkrng, ttot, in_maps = _prepare(inputs)
        nc = _build(budgets, blkrng, ttot)
        nc.compile()
        _cache.update(nc=nc, in_maps=in_maps)
    res = run_bass_kernel_spmd(_cache["nc"], _cache["in_maps"],
                               list(range(NCORES)))
    _cache["last_results"] = res
    return np.concatenate([res.results[c]["out"] for c in range(NCORES)], 0)
